# revision 4
# baseline (speedup 1.0000x reference)
"""AttnBlock (GroupNorm -> 1x1 qkv conv -> full attention -> 1x1 proj -> residual)
for x[8, 256, 64, 64] fp32, data-parallel over batch on 8 NeuronCores.

Per-core plan (one image, c=256 channels, n=4096 tokens), fp8 fast path:
  - GroupNorm(32 groups of 8 channels): per-channel bn_stats/bn_aggr along the
    free axis, tiny fp32 indicator matmul to reduce/broadcast across the
    8-partition channel groups; normalization folded to h = x*alpha + beta,
    written directly as fp8e4 (h ~ N(0,1), quantization rel err ~3%).
  - All big matmuls (QKV, S=K^T Q, PV, proj) run in fp8e4 with
    MatmulPerfMode.DoubleRow: one matmul contracts 2x128 channels (the fp8
    double-pumped PE path, ~2x bf16 throughput). Weights are pre-quantized to
    fp8 on the host. q/k/v get their fp32 biases added on ACT/Pool straight
    out of PSUM and are requantized to fp8.
  - Scores: exp((q8.k8)/16 - 4) computed on ACT per pair of key chunks (one
    [128, 1024] PSUM read spanning 2 banks), output fp8e4 e-values (max ~52,
    well under the 240 fp8e4 limit; the -4 shift guards overflow to s=9.5
    sigma). Softmax normalizer Z accumulates the fp8 e-values on the Pool
    engine (11/16 pairs) and DVE (4/16, disjoint accumulators) in fp32, so the
    e-quantization largely divides out through Z.
  - Z partition-reduce / broadcast via fp32r ones-matmuls; 16/Z broadcast is
    folded into the ones row (S_O=16 rescales attn out to ~unit std before
    fp8 requantization; the proj epilogue multiplies by 1/16).
  - Per 512-query block pipeline: S pairs run 2 ahead of exp; PV lags 6 pairs
    so the previous block's epilogue (Z chain + proj) injects into the PE
    stream while its PSUM accumulators drain; PSUM = 2x2-bank S pairs +
    2x1-bank PV accumulators + 2x1-bank epilogue rotation = 8 banks.
  - Residual comes from pre-filling out DRAM with x and accumulating proj
    results via DMA accum_op=add (last block fuses x on DVE + plain DMA).

Expected accuracy (numpy-emulated fp8 end to end): rel err ~6e-3.
"""

import contextlib
import ctypes
import os
import sys
import types

import ml_dtypes
import numpy as np

import concourse.tile as tile
from concourse import bacc, mybir
from concourse.bass_utils import run_bass_kernel_spmd


def _ensure_ntff_hook() -> bool:
    """Install an antenv.axon_hooks shim backed by libaxon_pjrt.so so that
    run_bass_kernel_spmd(trace=True) can capture NTFF profiles under axon.
    Returns True when tracing is possible."""
    try:
        from antenv.axon_hooks import get_axon_ntff_profile_hook  # noqa: F401

        return True
    except ImportError:
        pass
    so_path = "/opt/axon/libaxon_pjrt.so"
    if not os.path.exists(so_path):
        return False
    try:
        lib = ctypes.CDLL(so_path)
        if not hasattr(lib, "axon_start_nrt_profile"):
            return False
        lib.axon_start_nrt_profile.argtypes = [
            ctypes.POINTER(ctypes.c_int64),
            ctypes.c_size_t,
        ]
        lib.axon_start_nrt_profile.restype = ctypes.c_int64
        lib.axon_stop_nrt_profile.argtypes = [ctypes.c_char_p]
        lib.axon_stop_nrt_profile.restype = ctypes.c_int64
    except OSError:
        return False

    @contextlib.contextmanager
    def _hook(output_dir, device_ids):
        import jax

        jax.devices()
        if device_ids:
            ids = (ctypes.c_int64 * len(device_ids))(*device_ids)
            rc = lib.axon_start_nrt_profile(ids, len(device_ids))
        else:
            rc = lib.axon_start_nrt_profile(None, 0)
        if rc != 0:
            raise RuntimeError(f"axon_start_nrt_profile rc={rc}")
        try:
            yield
        finally:
            n = lib.axon_stop_nrt_profile(str(output_dir).encode())
            print(f"profile: {n} file(s) written to {output_dir}", file=sys.stderr)

    mod = types.ModuleType("antenv.axon_hooks")
    _state = {"hook": _hook}
    mod.get_axon_ntff_profile_hook = lambda: _state["hook"]
    mod.set_axon_ntff_profile_hook = lambda h: _state.__setitem__("hook", h)
    sys.modules["antenv.axon_hooks"] = mod
    import antenv

    antenv.axon_hooks = mod
    return True

F32 = mybir.dt.float32
F32R = mybir.dt.float32r
F8 = mybir.dt.float8e4
AX = mybir.AluOpType
AF = mybir.ActivationFunctionType
DR = mybir.MatmulPerfMode.DoubleRow

C = 256          # channels
N = 4096         # tokens (64*64)
P = 128          # partitions
CO = 2           # channel chunks (C // P)
QB = 512         # queries per block
NQB = N // QB    # 8 query blocks
NPAIR = N // (2 * P)  # 16 key-chunk pairs
EPS = 1e-5
SHIFT = 4.0      # exp(s/16 - SHIFT): e-values in (0, ~52], fp8e4-safe
SSCALE = 1.0 / 16.0
S_O = 16.0       # attn-out rescale before fp8 (folded into 1/Z broadcast)
INV_SO = 1.0 / S_O
# pairs whose Z-accumulation runs on DVE (rest on Pool); disjoint accumulators
DVE_PAIRS = (3, 7, 11, 15)

_LAST_RESULTS = None


def _build_program():
    nc = bacc.Bacc("TRN2", target_bir_lowering=False, debug=False, num_devices=8)

    x_d = nc.dram_tensor("x", [C, N], F32, kind="ExternalInput").ap()
    wqkT_d = nc.dram_tensor("wqkT", [C, 3 * C], F8, kind="ExternalInput").ap()
    bq_d = nc.dram_tensor("bq", [C], F32, kind="ExternalInput").ap()
    bk_d = nc.dram_tensor("bk", [C], F32, kind="ExternalInput").ap()
    bv_d = nc.dram_tensor("bv", [C], F32R, kind="ExternalInput").ap()
    projT_d = nc.dram_tensor("projT", [C, C], F8, kind="ExternalInput").ap()
    pb_d = nc.dram_tensor("pb", [C], F32, kind="ExternalInput").ap()
    nw_d = nc.dram_tensor("nw", [C], F32, kind="ExternalInput").ap()
    nb_d = nc.dram_tensor("nb", [C], F32, kind="ExternalInput").ap()
    gh_d = nc.dram_tensor("ghmat", [P, P], F32, kind="ExternalInput").ap()
    ones128_d = nc.dram_tensor("ones128", [P, 1], F32R, kind="ExternalInput").ap()
    ones1_d = nc.dram_tensor("ones1", [1, P], F32R, kind="ExternalInput").ap()
    srow_d = nc.dram_tensor("srow", [1, P], F32R, kind="ExternalInput").ap()
    out_d = nc.dram_tensor("out", [C, N], F32, kind="ExternalOutput").ap()

    # channel c = o*128 + p  ->  [partition, chunk, free]
    x_v = x_d.rearrange("(o p) m -> p o m", p=P)
    wqkT_v = wqkT_d.rearrange("(o p) m -> p o m", p=P)
    projT_v = projT_d.rearrange("(o p) m -> p o m", p=P)
    out_v = out_d.rearrange("(o p) m -> p o m", p=P)

    with tile.TileContext(nc) as tc:
        with (
            tc.tile_pool(name="cpool", bufs=1) as cpool,
            tc.tile_pool(name="bigs", bufs=1) as bigs,
            tc.tile_pool(name="epool", bufs=10) as epool,
            tc.tile_pool(name="zpool", bufs=2) as zpool,
            tc.tile_pool(name="spool", bufs=1) as spool,
            tc.tile_pool(name="wpool", bufs=2) as wpool,
            tc.tile_pool(name="psA", bufs=2, space="PSUM") as psA,
        ):
            # ---- input loads: x split into 8 chunks so stats/warmup overlap ----
            x_sb = bigs.tile([P, CO, N], F32)
            for co in range(CO):
                for c in range(4):
                    csl = slice(c * 1024, (c + 1) * 1024)
                    nc.sync.dma_start(out=x_sb[:, co, csl], in_=x_v[:, co, csl])
            wqk8 = cpool.tile([P, CO, 3 * C], F8)
            nc.sync.dma_start(out=wqk8, in_=wqkT_v)
            projT8 = cpool.tile([P, CO, C], F8)
            nc.sync.dma_start(out=projT8, in_=projT_v)

            def vec_tile(name, d_ap):
                t = cpool.tile([P, CO], F32, name=name)
                nc.sync.dma_start(out=t, in_=d_ap.rearrange("(o p) -> p o", p=P))
                return t

            bq_sb = vec_tile("bq_sb", bq_d)
            bk_sb = vec_tile("bk_sb", bk_d)
            pb_sb = vec_tile("pb_sb", pb_d)
            nw_sb = vec_tile("nw_sb", nw_d)
            nb_sb = vec_tile("nb_sb", nb_d)
            bv_row = cpool.tile([1, C], F32R)
            nc.sync.dma_start(out=bv_row, in_=bv_d.unsqueeze(0))
            gh_sb = cpool.tile([P, P], F32)
            nc.sync.dma_start(out=gh_sb, in_=gh_d)
            ones128 = cpool.tile([P, 1], F32R)
            nc.sync.dma_start(out=ones128, in_=ones128_d)
            ones1 = cpool.tile([1, P], F32R)
            nc.sync.dma_start(out=ones1, in_=ones1_d)
            srow = cpool.tile([1, P], F32R)
            nc.sync.dma_start(out=srow, in_=srow_d)
            eps_t = cpool.tile([P, 1], F32)
            nc.vector.memset(eps_t, EPS)
            shift_t = cpool.tile([P, 1], F32)
            nc.vector.memset(shift_t, -SHIFT)

            # ---- GroupNorm stats (per-channel along free axis) ----
            stats = spool.tile([P, CO, 8, 6], F32)
            mv = spool.tile([P, CO, 2], F32)
            for co in range(CO):
                for s in range(8):
                    nc.vector.bn_stats(
                        out=stats[:, co, s, :],
                        in_=x_sb[:, co, s * 512 : (s + 1) * 512],
                    )
                nc.vector.bn_aggr(out=mv[:, co, :], in_=stats[:, co])
            # rstats cols: [mean_co0, mean_co1, ex2_co0, ex2_co1]
            rstats = spool.tile([P, 4], F32)
            nc.vector.tensor_copy(out=rstats[:, 0:2], in_=mv[:, :, 0])
            nc.vector.tensor_tensor(
                out=rstats[:, 2:4], in0=mv[:, :, 0], in1=mv[:, :, 0], op=AX.mult)
            nc.vector.tensor_tensor(
                out=rstats[:, 2:4], in0=rstats[:, 2:4], in1=mv[:, :, 1], op=AX.add)
            # group mean over 8 adjacent partitions, broadcast back, in one
            # block-diagonal (1/8) indicator matmul (fp32 exact)
            bps = psA.tile([P, 4], F32, tag="epi", name="bps")
            nc.tensor.matmul(bps, lhsT=gh_sb, rhs=rstats, start=True, stop=True)
            bss = spool.tile([P, 4], F32)
            nc.vector.tensor_copy(out=bss, in_=bps)
            # var = ex2 - mu^2 ; rstd = 1/sqrt(var + eps)
            var = spool.tile([P, 2], F32)
            nc.vector.tensor_tensor(out=var, in0=bss[:, 0:2], in1=bss[:, 0:2], op=AX.mult)
            nc.vector.tensor_tensor(out=var, in0=bss[:, 2:4], in1=var, op=AX.subtract)
            sd = spool.tile([P, 2], F32)
            nc.scalar.activation(out=sd, in_=var, func=AF.Sqrt, bias=eps_t, scale=1.0)
            rstd = spool.tile([P, 2], F32)
            nc.vector.reciprocal(out=rstd, in_=sd)
            alpha = spool.tile([P, 2], F32)
            nc.vector.tensor_tensor(out=alpha, in0=rstd, in1=nw_sb, op=AX.mult)
            beta = spool.tile([P, 2], F32)
            nc.vector.tensor_tensor(out=beta, in0=bss[:, 0:2], in1=alpha, op=AX.mult)
            nc.vector.tensor_tensor(out=beta, in0=nb_sb, in1=beta, op=AX.subtract)

            # residual pre-fill: out <- x; proj results are DMA-accumulated later
            for co in range(CO):
                nc.sync.dma_start(out=out_v[:, co, :], in_=x_sb[:, co, :])

            # broadcast V bias row across partitions once: bvb[p, c] = bv[c]
            bvb_ps = psA.tile([P, C], F32, tag="epi", name="bvb_ps")
            nc.tensor.matmul(bvb_ps, lhsT=ones1, rhs=bv_row, start=True, stop=True)
            bvb = cpool.tile([P, C], F32)
            nc.vector.tensor_copy(out=bvb, in_=bvb_ps)

            # ---- h (fp8) + QKV via DoubleRow fp8 matmuls, per 512-token block ----
            hn8 = bigs.tile([P, CO, N], F8)
            q8 = bigs.tile([P, CO, N], F8)
            k8 = bigs.tile([P, CO, N], F8)
            vt8 = bigs.tile([P, 2 * NPAIR, C], F8)

            for blk in range(NQB):
                sl = slice(blk * QB, (blk + 1) * QB)
                for co in range(CO):
                    nc.vector.tensor_scalar(
                        out=hn8[:, co, sl], in0=x_sb[:, co, sl],
                        scalar1=alpha[:, co : co + 1], scalar2=beta[:, co : co + 1],
                        op0=AX.mult, op1=AX.add,
                    )
                rot = 0
                for dst8, bias_sb, off in ((q8, bq_sb, 0), (k8, bk_sb, C)):
                    for cout in range(CO):
                        ps = psA.tile(
                            [P, QB], F32, tag=("pv" if rot % 2 == 0 else "epi"),
                            name="qk_ps")
                        rot += 1
                        nc.tensor.matmul(
                            ps,
                            lhsT=wqk8[:, :, off + cout * P : off + (cout + 1) * P],
                            rhs=hn8[:, :, sl],
                            start=True, stop=True, perf_mode=DR,
                        )
                        nc.scalar.activation(
                            out=dst8[:, cout, sl], in_=ps,
                            func=AF.Identity, bias=bias_sb[:, cout : cout + 1],
                            scale=1.0,
                        )
                for ko in range(4 * blk, 4 * blk + 4):
                    ps = psA.tile(
                        [P, C], F32, tag=("pv" if rot % 2 == 0 else "epi"),
                        name="vt_ps")
                    rot += 1
                    nc.tensor.matmul(
                        ps,
                        lhsT=hn8[:, :, ko * P : (ko + 1) * P],
                        rhs=wqk8[:, :, 2 * C : 3 * C],
                        start=True, stop=True, perf_mode=DR,
                    )
                    nc.vector.tensor_tensor(
                        out=vt8[:, ko, :], in0=ps, in1=bvb, op=AX.add)

            # ---- attention (pipelined; prev block epilogue injected) ----
            out8 = bigs.tile([P, CO, N], F8)

            def make_block(qb):
                ctx = {"qb": qb, "es": [None] * NPAIR}
                ctx["pso"] = [
                    psA.tile([P, QB], F32, tag="pv", name=f"pso{cc}")
                    for cc in range(CO)
                ]
                ctx["zP"] = zpool.tile([P, 2 * QB], F32R, tag="zP", name="zP")
                ctx["zD"] = zpool.tile([P, 2 * QB], F32R, tag="zD", name="zD")
                return ctx

            def do_spair(ctx, j):
                qsl = slice(ctx["qb"] * QB, (ctx["qb"] + 1) * QB)
                ps2 = psA.tile([P, 2 * QB], F32, tag="spair", name="s_ps2")
                for t in range(2):
                    ch = 2 * j + t
                    nc.tensor.matmul(
                        ps2[:, t * QB : (t + 1) * QB],
                        lhsT=k8[:, :, ch * P : (ch + 1) * P],
                        rhs=q8[:, :, qsl],
                        start=True, stop=True, perf_mode=DR,
                    )
                e2 = epool.tile([P, 2 * QB], F8, name="e_pair")
                nc.scalar.activation(
                    out=e2, in_=ps2, func=AF.Exp, bias=shift_t, scale=SSCALE)
                ctx["es"][j] = e2

            def do_zadd(ctx, j):
                es = ctx["es"]
                if j == 1:
                    nc.gpsimd.tensor_tensor(
                        out=ctx["zP"], in0=es[0], in1=es[1], op=AX.add)
                elif j == 7:
                    nc.vector.tensor_tensor(
                        out=ctx["zD"], in0=es[3], in1=es[7], op=AX.add)
                elif j in (11, 15):
                    nc.vector.tensor_tensor(
                        out=ctx["zD"], in0=ctx["zD"], in1=es[j], op=AX.add)
                elif j not in (0, 3):
                    nc.gpsimd.tensor_tensor(
                        out=ctx["zP"], in0=ctx["zP"], in1=es[j], op=AX.add)

            def do_pv(ctx, j):
                e3 = ctx["es"][j].rearrange("p (a b) -> p a b", a=2)
                for cc in range(CO):
                    nc.tensor.matmul(
                        ctx["pso"][cc],
                        lhsT=vt8[:, 2 * j : 2 * j + 2, cc * P : (cc + 1) * P],
                        rhs=e3,
                        start=(j == 0), stop=(j == NPAIR - 1), perf_mode=DR,
                    )

            def epi_zfold(ctx):
                zaF = wpool.tile([P, QB], F32R, tag="wz", name="zaF")
                nc.gpsimd.tensor_tensor(
                    out=zaF, in0=ctx["zP"][:, :QB], in1=ctx["zP"][:, QB:], op=AX.add)
                nc.gpsimd.tensor_tensor(
                    out=zaF, in0=zaF, in1=ctx["zD"][:, :QB], op=AX.add)
                nc.gpsimd.tensor_tensor(
                    out=zaF, in0=zaF, in1=ctx["zD"][:, QB:], op=AX.add)
                ctx["zaF"] = zaF

            def epi_zsum(ctx):
                zps = psA.tile([1, QB], F32, tag="epi", name="zps")
                nc.tensor.matmul(
                    zps, lhsT=ones128, rhs=ctx["zaF"], start=True, stop=True)
                ctx["zps"] = zps

            def epi_recip(ctx):
                zr = wpool.tile([1, QB], F32R, tag="wr", name="zr")
                with nc.allow_low_precision(reason="1/Z rounded to fp22 once"):
                    nc.vector.reciprocal(out=zr, in_=ctx["zps"])
                ctx["zr"] = zr

            def epi_zbp(ctx):
                # broadcast S_O/Z to all partitions (srow holds S_O)
                zbp = psA.tile([P, QB], F32, tag="epi", name="zbp")
                nc.tensor.matmul(
                    zbp, lhsT=srow, rhs=ctx["zr"], start=True, stop=True)
                ctx["zbp"] = zbp

            def epi_zbs(ctx):
                zbs = wpool.tile([P, QB], F32, tag="wb", name="zbs")
                nc.vector.tensor_copy(out=zbs, in_=ctx["zbp"])
                ctx["zbs"] = zbs

            def epi_outsc(ctx):
                qb = ctx["qb"]
                for cc in range(CO):
                    nc.vector.tensor_tensor(
                        out=out8[:, cc, qb * QB : (qb + 1) * QB],
                        in0=ctx["pso"][cc], in1=ctx["zbs"], op=AX.mult,
                    )

            def epi_proj(ctx, cout, last=False):
                qb = ctx["qb"]
                sl = slice(qb * QB, (qb + 1) * QB)
                pj = psA.tile([P, QB], F32, tag="epi", name="pj")
                nc.tensor.matmul(
                    pj,
                    lhsT=projT8[:, :, cout * P : (cout + 1) * P],
                    rhs=out8[:, :, sl],
                    start=True, stop=True, perf_mode=DR,
                )
                fin = wpool.tile([P, QB], F32, tag="wf", name="fin")
                if last:
                    nc.vector.scalar_tensor_tensor(
                        out=fin, in0=pj, scalar=INV_SO,
                        in1=ctx["xpb"][:, cout, :], op0=AX.mult, op1=AX.add,
                    )
                    nc.sync.dma_start(out=out_v[:, cout, sl], in_=fin)
                else:
                    nc.vector.tensor_scalar(
                        out=fin, in0=pj, scalar1=INV_SO,
                        scalar2=pb_sb[:, cout : cout + 1],
                        op0=AX.mult, op1=AX.add,
                    )
                    nc.gpsimd.dma_start(
                        out=out_v[:, cout, sl], in_=fin, accum_op=AX.add)

            def inject(prev, j):
                if prev is None:
                    return
                if j == 0:
                    epi_zfold(prev)
                elif j == 1:
                    epi_zsum(prev)
                elif j == 2:
                    epi_recip(prev)
                elif j == 3:
                    epi_zbp(prev)
                elif j == 4:
                    epi_zbs(prev)
                elif j == 5:
                    epi_outsc(prev)
                elif j == 7:
                    epi_proj(prev, 0)
                elif j == 9:
                    epi_proj(prev, 1)

            prev = None
            for qb in range(NQB):
                ctx = make_block(qb)
                if qb == NQB - 1:
                    # x + pb pre-sum for the fused last-block residual
                    lsl = slice(qb * QB, (qb + 1) * QB)
                    xpb = spool.tile([P, CO, QB], F32, name="xpb")
                    for cc in range(CO):
                        nc.vector.tensor_scalar(
                            out=xpb[:, cc, :], in0=x_sb[:, cc, lsl],
                            scalar1=pb_sb[:, cc : cc + 1], scalar2=None,
                            op0=AX.add,
                        )
                    ctx["xpb"] = xpb
                do_spair(ctx, 0)
                do_spair(ctx, 1)
                for j in range(NPAIR):
                    if j + 2 < NPAIR:
                        do_spair(ctx, j + 2)
                    do_zadd(ctx, j)
                    inject(prev, j)
                    if j >= 6:
                        do_pv(ctx, j - 6)
                for j in range(NPAIR - 6, NPAIR):
                    do_pv(ctx, j)
                prev = ctx
            # tail: last block epilogue
            epi_zfold(prev)
            epi_zsum(prev)
            epi_recip(prev)
            epi_zbp(prev)
            epi_zbs(prev)
            epi_outsc(prev)
            epi_proj(prev, 0, last=True)
            epi_proj(prev, 1, last=True)

    nc.compile()
    return nc


def _host_inputs(x, norm_w, norm_b, qkv_w, qkv_b, proj_w, proj_b):
    f = np.float32
    f8 = ml_dtypes.float8_e4m3
    wqkT = np.ascontiguousarray(qkv_w.T).astype(f).astype(f8)   # [c_in, 3C] fp8
    projT = np.ascontiguousarray(proj_w.T).astype(f).astype(f8)
    gh = np.zeros((P, P), f)
    gh[np.arange(P)[:, None] // 8 == np.arange(P)[None, :] // 8] = 0.125
    shared = {
        "wqkT": wqkT,
        "bq": qkv_b[:C].astype(f),
        "bk": qkv_b[C : 2 * C].astype(f),
        "bv": qkv_b[2 * C : 3 * C].astype(f),
        "projT": projT, "pb": proj_b.astype(f),
        "nw": norm_w.astype(f), "nb": norm_b.astype(f),
        "ghmat": gh,
        "ones128": np.ones((P, 1), f), "ones1": np.ones((1, P), f),
        "srow": np.full((1, P), S_O, f),
    }
    xs = np.ascontiguousarray(x.reshape(x.shape[0], C, N).astype(f))
    return [dict(shared, x=xs[i]) for i in range(x.shape[0])]


def kernel(x, norm_w, norm_b, qkv_w, qkv_b, proj_w, proj_b):
    global _LAST_RESULTS
    B = x.shape[0]
    nc = _build_program()
    in_maps = _host_inputs(x, norm_w, norm_b, qkv_w, qkv_b, proj_w, proj_b)
    trace = bool(int(os.environ.get("KERNEL_TRACE", "0"))) or bool(
        os.environ.get("BASS_TRACE")
    )
    if trace:
        trace = _ensure_ntff_hook()
    res = run_bass_kernel_spmd(
        nc, in_maps, core_ids=list(range(B)), trace=trace,
    )
    _LAST_RESULTS = res
    out = np.stack([res.results[i]["out"] for i in range(B)])
    return out.reshape(B, C, 64, 64)


# revision 10
# speedup vs baseline: 1.3739x; 1.3739x over previous
"""AttnBlock (GroupNorm -> 1x1 qkv conv -> full attention -> 1x1 proj -> residual)
for x[8, 256, 64, 64] fp32, data-parallel over batch on 8 NeuronCores.

Per-core plan (one image, c=256 channels, n=4096 tokens), fp8 fast path:
  - GroupNorm(32 groups of 8 channels): per-channel bn_stats/bn_aggr along the
    free axis, tiny fp32 indicator matmul to reduce/broadcast across the
    8-partition channel groups; normalization folded to h = x*alpha + beta,
    written directly as fp8e4 (h ~ N(0,1), quantization rel err ~3%).
  - All big matmuls (QKV, S=K^T Q, PV, proj) run in fp8e4 with
    MatmulPerfMode.DoubleRow: one matmul contracts 2x128 channels (the fp8
    double-pumped PE path, ~2x bf16 column rate). Weights are pre-quantized
    to fp8 on the host; V's bias is folded into the proj bias on the host
    (softmax weights sum to 1, so proj_w @ bv is an exact constant shift).
  - Scores: exp((q8.k8)/16 - 4) on ACT per pair of key chunks (one
    [128, 1024] PSUM read spanning 2 banks), fp8e4 e-values (max ~52 << 240).
  - Softmax normalizer Z accumulates ON THE PE: per pair one extra DoubleRow
    matmul with a ones[128,2,1] stationary accumulates Z into a [1,512] PSUM
    row across all 16 pairs (fp32-exact, ~213ns/pair) -- the elementwise
    engines are far too slow for the 16.8M adds (measured).
  - 16/Z broadcast via an fp32r ones-row matmul (S_O=16 rescales attn out to
    ~unit std before fp8 requantization; the proj epilogue multiplies 1/16).
  - Per 512-query block pipeline: S pairs run 2 ahead of exp, the Z matmul
    2 behind, PV 5 behind (so the previous block's epilogue, injected at the
    block head, drains the PV accumulators before they are reused).
    PSUM = 2x2-bank S pairs + 2 PV banks + 1 epilogue bank + 1 Z bank = 8.
  - Residual comes from pre-filling out DRAM with x and accumulating proj
    results via DMA accum_op=add (last block fuses x on DVE + plain DMA).

Measured accuracy: rel err ~5e-3 (gate 2e-2).
"""

import contextlib
import ctypes
import os
import sys
import types

import ml_dtypes
import numpy as np

import concourse.tile as tile
from concourse import bacc, mybir
from concourse.bass_utils import run_bass_kernel_spmd


def _ensure_ntff_hook() -> bool:
    """Install an antenv.axon_hooks shim backed by libaxon_pjrt.so so that
    run_bass_kernel_spmd(trace=True) can capture NTFF profiles under axon.
    Returns True when tracing is possible."""
    try:
        from antenv.axon_hooks import get_axon_ntff_profile_hook  # noqa: F401

        return True
    except ImportError:
        pass
    so_path = "/opt/axon/libaxon_pjrt.so"
    if not os.path.exists(so_path):
        return False
    try:
        lib = ctypes.CDLL(so_path)
        if not hasattr(lib, "axon_start_nrt_profile"):
            return False
        lib.axon_start_nrt_profile.argtypes = [
            ctypes.POINTER(ctypes.c_int64),
            ctypes.c_size_t,
        ]
        lib.axon_start_nrt_profile.restype = ctypes.c_int64
        lib.axon_stop_nrt_profile.argtypes = [ctypes.c_char_p]
        lib.axon_stop_nrt_profile.restype = ctypes.c_int64
    except OSError:
        return False

    @contextlib.contextmanager
    def _hook(output_dir, device_ids):
        import jax

        jax.devices()
        if device_ids:
            ids = (ctypes.c_int64 * len(device_ids))(*device_ids)
            rc = lib.axon_start_nrt_profile(ids, len(device_ids))
        else:
            rc = lib.axon_start_nrt_profile(None, 0)
        if rc != 0:
            raise RuntimeError(f"axon_start_nrt_profile rc={rc}")
        try:
            yield
        finally:
            n = lib.axon_stop_nrt_profile(str(output_dir).encode())
            print(f"profile: {n} file(s) written to {output_dir}", file=sys.stderr)

    mod = types.ModuleType("antenv.axon_hooks")
    _state = {"hook": _hook}
    mod.get_axon_ntff_profile_hook = lambda: _state["hook"]
    mod.set_axon_ntff_profile_hook = lambda h: _state.__setitem__("hook", h)
    sys.modules["antenv.axon_hooks"] = mod
    import antenv

    antenv.axon_hooks = mod
    return True

F32 = mybir.dt.float32
F32R = mybir.dt.float32r
F8 = mybir.dt.float8e4
AX = mybir.AluOpType
AF = mybir.ActivationFunctionType
DR = mybir.MatmulPerfMode.DoubleRow

C = 256          # channels
N = 4096         # tokens (64*64)
P = 128          # partitions
CO = 2           # channel chunks (C // P)
QB = 512         # queries per block
NQB = N // QB    # 8 query blocks
NPAIR = N // (2 * P)  # 16 key-chunk pairs
EPS = 1e-5
SHIFT = 4.0      # exp(s/16 - SHIFT): e-values in (0, ~52], fp8e4-safe
SSCALE = 1.0 / 16.0
S_O = 16.0       # attn-out rescale before fp8 (folded into 1/Z broadcast)
INV_SO = 1.0 / S_O

_LAST_RESULTS = None


def _build_program():
    nc = bacc.Bacc("TRN2", target_bir_lowering=False, debug=False, num_devices=8)

    x_d = nc.dram_tensor("x", [C, N], F32, kind="ExternalInput").ap()
    wqkT_d = nc.dram_tensor("wqkT", [C, 3 * C], F8, kind="ExternalInput").ap()
    bq_d = nc.dram_tensor("bq", [C], F32, kind="ExternalInput").ap()
    bk_d = nc.dram_tensor("bk", [C], F32, kind="ExternalInput").ap()
    projT_d = nc.dram_tensor("projT", [C, C], F8, kind="ExternalInput").ap()
    pb_d = nc.dram_tensor("pb", [C], F32, kind="ExternalInput").ap()
    nw_d = nc.dram_tensor("nw", [C], F32, kind="ExternalInput").ap()
    nb_d = nc.dram_tensor("nb", [C], F32, kind="ExternalInput").ap()
    gh_d = nc.dram_tensor("ghmat", [P, P], F32, kind="ExternalInput").ap()
    srow_d = nc.dram_tensor("srow", [1, P], F32R, kind="ExternalInput").ap()
    out_d = nc.dram_tensor("out", [C, N], F32, kind="ExternalOutput").ap()

    # channel c = o*128 + p  ->  [partition, chunk, free]
    x_v = x_d.rearrange("(o p) m -> p o m", p=P)
    wqkT_v = wqkT_d.rearrange("(o p) m -> p o m", p=P)
    projT_v = projT_d.rearrange("(o p) m -> p o m", p=P)
    out_v = out_d.rearrange("(o p) m -> p o m", p=P)

    with tile.TileContext(nc) as tc:
        with (
            tc.tile_pool(name="cpool", bufs=1) as cpool,
            tc.tile_pool(name="bigs", bufs=1) as bigs,
            tc.tile_pool(name="epool", bufs=10) as epool,
            tc.tile_pool(name="spool", bufs=1) as spool,
            tc.tile_pool(name="wpool", bufs=2) as wpool,
            tc.tile_pool(name="psA", bufs=2, space="PSUM") as psA,
        ):
            # ---- input loads: x split into 8 chunks so stats/warmup overlap ----
            x_sb = bigs.tile([P, CO, N], F32)
            for co in range(CO):
                for c in range(4):
                    csl = slice(c * 1024, (c + 1) * 1024)
                    nc.sync.dma_start(out=x_sb[:, co, csl], in_=x_v[:, co, csl])
            wqk8 = cpool.tile([P, CO, 3 * C], F8)
            nc.sync.dma_start(out=wqk8, in_=wqkT_v)
            projT8 = cpool.tile([P, CO, C], F8)
            nc.sync.dma_start(out=projT8, in_=projT_v)

            def vec_tile(name, d_ap):
                t = cpool.tile([P, CO], F32, name=name)
                nc.sync.dma_start(out=t, in_=d_ap.rearrange("(o p) -> p o", p=P))
                return t

            bq_sb = vec_tile("bq_sb", bq_d)
            bk_sb = vec_tile("bk_sb", bk_d)
            pb_sb = vec_tile("pb_sb", pb_d)
            nw_sb = vec_tile("nw_sb", nw_d)
            nb_sb = vec_tile("nb_sb", nb_d)
            gh_sb = cpool.tile([P, P], F32)
            nc.sync.dma_start(out=gh_sb, in_=gh_d)
            # [P, 2, 16] so the DoubleRow stationary AP's pair-stride is 16B
            # (s3_lw_dual_fp8_restrictions: step%16==0); only column 0 is used
            ones8t = cpool.tile([P, 2, 16], F8)
            nc.vector.memset(ones8t, 1.0)
            srow = cpool.tile([1, P], F32R)
            nc.sync.dma_start(out=srow, in_=srow_d)
            eps_t = cpool.tile([P, 1], F32)
            nc.vector.memset(eps_t, EPS)
            shift_t = cpool.tile([P, 1], F32)
            nc.vector.memset(shift_t, -SHIFT)

            # ---- GroupNorm stats (per-channel along free axis) ----
            stats = spool.tile([P, CO, 8, 6], F32)
            mv = spool.tile([P, CO, 2], F32)
            for co in range(CO):
                for s in range(8):
                    nc.vector.bn_stats(
                        out=stats[:, co, s, :],
                        in_=x_sb[:, co, s * 512 : (s + 1) * 512],
                    )
                nc.vector.bn_aggr(out=mv[:, co, :], in_=stats[:, co])
            # rstats cols: [mean_co0, mean_co1, ex2_co0, ex2_co1]
            rstats = spool.tile([P, 4], F32)
            nc.vector.tensor_copy(out=rstats[:, 0:2], in_=mv[:, :, 0])
            nc.vector.tensor_tensor(
                out=rstats[:, 2:4], in0=mv[:, :, 0], in1=mv[:, :, 0], op=AX.mult)
            nc.vector.tensor_tensor(
                out=rstats[:, 2:4], in0=rstats[:, 2:4], in1=mv[:, :, 1], op=AX.add)
            # group mean over 8 adjacent partitions, broadcast back, in one
            # block-diagonal (1/8) indicator matmul (fp32 exact)
            bps = psA.tile([P, 4], F32, tag="epi", bufs=1, name="bps")
            nc.tensor.matmul(bps, lhsT=gh_sb, rhs=rstats, start=True, stop=True)
            bss = spool.tile([P, 4], F32)
            nc.vector.tensor_copy(out=bss, in_=bps)
            # var = ex2 - mu^2 ; rstd = 1/sqrt(var + eps)
            var = spool.tile([P, 2], F32)
            nc.vector.tensor_tensor(out=var, in0=bss[:, 0:2], in1=bss[:, 0:2], op=AX.mult)
            nc.vector.tensor_tensor(out=var, in0=bss[:, 2:4], in1=var, op=AX.subtract)
            sd = spool.tile([P, 2], F32)
            nc.scalar.activation(out=sd, in_=var, func=AF.Sqrt, bias=eps_t, scale=1.0)
            rstd = spool.tile([P, 2], F32)
            nc.vector.reciprocal(out=rstd, in_=sd)
            alpha = spool.tile([P, 2], F32)
            nc.vector.tensor_tensor(out=alpha, in0=rstd, in1=nw_sb, op=AX.mult)
            beta = spool.tile([P, 2], F32)
            nc.vector.tensor_tensor(out=beta, in0=bss[:, 0:2], in1=alpha, op=AX.mult)
            nc.vector.tensor_tensor(out=beta, in0=nb_sb, in1=beta, op=AX.subtract)

            # residual pre-fill: out <- x; proj results are DMA-accumulated later
            for co in range(CO):
                nc.sync.dma_start(out=out_v[:, co, :], in_=x_sb[:, co, :])

            # ---- h (fp8) + QKV via DoubleRow fp8 matmuls, per 512-token block ----
            hn8 = bigs.tile([P, CO, N], F8)
            q8 = bigs.tile([P, CO, N], F8)
            k8 = bigs.tile([P, CO, N], F8)
            vt8 = bigs.tile([P, 2 * NPAIR, C], F8)

            for blk in range(NQB):
                sl = slice(blk * QB, (blk + 1) * QB)
                for co in range(CO):
                    nc.vector.tensor_scalar(
                        out=hn8[:, co, sl], in0=x_sb[:, co, sl],
                        scalar1=alpha[:, co : co + 1], scalar2=beta[:, co : co + 1],
                        op0=AX.mult, op1=AX.add,
                    )
                rot = 0
                for dst8, bias_sb, off in ((q8, bq_sb, 0), (k8, bk_sb, C)):
                    for cout in range(CO):
                        ps = psA.tile(
                            [P, QB], F32, tag=("pv" if rot % 2 == 0 else "spair"),
                            name="qk_ps")
                        rot += 1
                        nc.tensor.matmul(
                            ps,
                            lhsT=wqk8[:, :, off + cout * P : off + (cout + 1) * P],
                            rhs=hn8[:, :, sl],
                            start=True, stop=True, perf_mode=DR,
                        )
                        nc.scalar.activation(
                            out=dst8[:, cout, sl], in_=ps,
                            func=AF.Identity, bias=bias_sb[:, cout : cout + 1],
                            scale=1.0,
                        )
                for ko in range(4 * blk, 4 * blk + 4):
                    ps = psA.tile(
                        [P, C], F32, tag=("pv" if rot % 2 == 0 else "spair"),
                        name="vt_ps")
                    rot += 1
                    nc.tensor.matmul(
                        ps,
                        lhsT=hn8[:, :, ko * P : (ko + 1) * P],
                        rhs=wqk8[:, :, 2 * C : 3 * C],
                        start=True, stop=True, perf_mode=DR,
                    )
                    nc.vector.tensor_copy(out=vt8[:, ko, :], in_=ps)

            # ---- attention (pipelined; prev block epilogue injected) ----
            out8 = bigs.tile([P, CO, N], F8)

            def make_block(qb):
                ctx = {"qb": qb, "es": [None] * NPAIR}
                ctx["pso"] = [
                    psA.tile([P, QB], F32, tag="pv", name=f"pso{cc}")
                    for cc in range(CO)
                ]
                ctx["zrow"] = psA.tile(
                    [1, QB], F32, tag="zrow", bufs=1, name="zrow")
                return ctx

            def do_spair(ctx, j):
                qsl = slice(ctx["qb"] * QB, (ctx["qb"] + 1) * QB)
                ps2 = psA.tile([P, 2 * QB], F32, tag="spair", name="s_ps2")
                for t in range(2):
                    ch = 2 * j + t
                    nc.tensor.matmul(
                        ps2[:, t * QB : (t + 1) * QB],
                        lhsT=k8[:, :, ch * P : (ch + 1) * P],
                        rhs=q8[:, :, qsl],
                        start=True, stop=True, perf_mode=DR,
                    )
                e2 = epool.tile([P, 2 * QB], F8, name="e_pair")
                nc.scalar.activation(
                    out=e2, in_=ps2, func=AF.Exp, bias=shift_t, scale=SSCALE)
                ctx["es"][j] = e2

            def do_z(ctx, j):
                e3 = ctx["es"][j].rearrange("p (a b) -> p a b", a=2)
                nc.tensor.matmul(
                    ctx["zrow"], lhsT=ones8t[:, :, 0:1], rhs=e3,
                    start=(j == 0), stop=(j == NPAIR - 1), perf_mode=DR,
                )

            def do_pv(ctx, j):
                e3 = ctx["es"][j].rearrange("p (a b) -> p a b", a=2)
                for cc in range(CO):
                    nc.tensor.matmul(
                        ctx["pso"][cc],
                        lhsT=vt8[:, 2 * j : 2 * j + 2, cc * P : (cc + 1) * P],
                        rhs=e3,
                        start=(j == 0), stop=(j == NPAIR - 1), perf_mode=DR,
                    )

            def epi_recip(ctx):
                zr = wpool.tile([1, QB], F32R, tag="wr", name="zr")
                with nc.allow_low_precision(reason="1/Z rounded to fp22 once"):
                    nc.vector.reciprocal(out=zr, in_=ctx["zrow"])
                ctx["zr"] = zr

            def epi_zbp(ctx):
                # broadcast S_O/Z to all partitions (srow holds S_O)
                zbp = psA.tile([P, QB], F32, tag="epi", bufs=1, name="zbp")
                nc.tensor.matmul(
                    zbp, lhsT=srow, rhs=ctx["zr"], start=True, stop=True)
                ctx["zbp"] = zbp

            def epi_zbs(ctx):
                zbs = wpool.tile([P, QB], F32, tag="wb", name="zbs")
                nc.vector.tensor_copy(out=zbs, in_=ctx["zbp"])
                ctx["zbs"] = zbs

            def epi_outsc(ctx, cc):
                qb = ctx["qb"]
                nc.vector.tensor_tensor(
                    out=out8[:, cc, qb * QB : (qb + 1) * QB],
                    in0=ctx["pso"][cc], in1=ctx["zbs"], op=AX.mult,
                )

            def epi_proj(ctx, cout, last=False):
                qb = ctx["qb"]
                sl = slice(qb * QB, (qb + 1) * QB)
                pj = psA.tile([P, QB], F32, tag="epi", bufs=1, name="pj")
                nc.tensor.matmul(
                    pj,
                    lhsT=projT8[:, :, cout * P : (cout + 1) * P],
                    rhs=out8[:, :, sl],
                    start=True, stop=True, perf_mode=DR,
                )
                fin = wpool.tile([P, QB], F32, tag="wf", name="fin")
                if last:
                    nc.vector.scalar_tensor_tensor(
                        out=fin, in0=pj, scalar=INV_SO,
                        in1=ctx["xpb"][:, cout, :], op0=AX.mult, op1=AX.add,
                    )
                    nc.sync.dma_start(out=out_v[:, cout, sl], in_=fin)
                else:
                    nc.vector.tensor_scalar(
                        out=fin, in0=pj, scalar1=INV_SO,
                        scalar2=pb_sb[:, cout : cout + 1],
                        op0=AX.mult, op1=AX.add,
                    )
                    nc.gpsimd.dma_start(
                        out=out_v[:, cout, sl], in_=fin, accum_op=AX.add)

            def inject(prev, j):
                if prev is None:
                    return
                if j == 0:
                    epi_recip(prev)
                elif j == 1:
                    epi_zbp(prev)
                elif j == 2:
                    epi_zbs(prev)
                elif j == 3:
                    epi_outsc(prev, 0)
                elif j == 4:
                    epi_outsc(prev, 1)
                elif j == 6:
                    epi_proj(prev, 0)
                elif j == 8:
                    epi_proj(prev, 1)

            prev = None
            for qb in range(NQB):
                ctx = make_block(qb)
                if qb == NQB - 1:
                    # x + pb pre-sum for the fused last-block residual
                    lsl = slice(qb * QB, (qb + 1) * QB)
                    xpb = spool.tile([P, CO, QB], F32, name="xpb")
                    for cc in range(CO):
                        nc.vector.tensor_scalar(
                            out=xpb[:, cc, :], in0=x_sb[:, cc, lsl],
                            scalar1=pb_sb[:, cc : cc + 1], scalar2=None,
                            op0=AX.add,
                        )
                    ctx["xpb"] = xpb
                do_spair(ctx, 0)
                do_spair(ctx, 1)
                for j in range(NPAIR):
                    if j + 2 < NPAIR:
                        do_spair(ctx, j + 2)
                    if j >= 2:
                        do_z(ctx, j - 2)
                    inject(prev, j)
                    if j >= 5:
                        do_pv(ctx, j - 5)
                do_z(ctx, NPAIR - 2)
                do_z(ctx, NPAIR - 1)
                for j in range(NPAIR - 5, NPAIR):
                    do_pv(ctx, j)
                prev = ctx
            # tail: last block epilogue
            epi_recip(prev)
            epi_zbp(prev)
            epi_zbs(prev)
            epi_outsc(prev, 0)
            epi_outsc(prev, 1)
            epi_proj(prev, 0, last=True)
            epi_proj(prev, 1, last=True)

    nc.compile()
    return nc


def _host_inputs(x, norm_w, norm_b, qkv_w, qkv_b, proj_w, proj_b):
    f = np.float32
    f8 = ml_dtypes.float8_e4m3
    wqkT = np.ascontiguousarray(qkv_w.T).astype(f).astype(f8)   # [c_in, 3C] fp8
    projT = np.ascontiguousarray(proj_w.T).astype(f).astype(f8)
    bv = qkv_b[2 * C : 3 * C].astype(np.float64)
    # softmax weights sum to 1, so the V bias contributes proj_w @ bv exactly
    pb_eff = (proj_b.astype(np.float64) + proj_w.astype(np.float64) @ bv).astype(f)
    gh = np.zeros((P, P), f)
    gh[np.arange(P)[:, None] // 8 == np.arange(P)[None, :] // 8] = 0.125
    shared = {
        "wqkT": wqkT,
        "bq": qkv_b[:C].astype(f),
        "bk": qkv_b[C : 2 * C].astype(f),
        "projT": projT, "pb": pb_eff,
        "nw": norm_w.astype(f), "nb": norm_b.astype(f),
        "ghmat": gh,
        "srow": np.full((1, P), S_O, f),
    }
    xs = np.ascontiguousarray(x.reshape(x.shape[0], C, N).astype(f))
    return [dict(shared, x=xs[i]) for i in range(x.shape[0])]


def kernel(x, norm_w, norm_b, qkv_w, qkv_b, proj_w, proj_b):
    global _LAST_RESULTS
    B = x.shape[0]
    nc = _build_program()
    in_maps = _host_inputs(x, norm_w, norm_b, qkv_w, qkv_b, proj_w, proj_b)
    trace = bool(int(os.environ.get("KERNEL_TRACE", "0"))) or bool(
        os.environ.get("BASS_TRACE")
    )
    if trace:
        trace = _ensure_ntff_hook()
    res = run_bass_kernel_spmd(
        nc, in_maps, core_ids=list(range(B)), trace=trace,
    )
    _LAST_RESULTS = res
    out = np.stack([res.results[i]["out"] for i in range(B)])
    return out.reshape(B, C, 64, 64)
